# revision 7
# baseline (speedup 1.0000x reference)
"""Window-routed sparse attention on 8 TRN2 NeuronCores.

Sharding: 64 windows x 8 cores = 8 windows/core (embarrassingly parallel).
Host precomputes the tiny routing path (region means, a_r [64,64]) and the
window-mixed q_m/k_m in fp32; each core runs the heavy windowed attention
relu(q_m k_m^T) v for its 8 windows on the Tensor engine in bf16 (f32 PSUM
accumulation). Relu alternates between the Scalar and Vector engines so it
hides under the matmuls; PSUM->SBUF output copies run on GpSimd.
"""

import sys

sys.path.insert(0, "/opt/trn_rl_repo")

import numpy as np
import ml_dtypes

C = 64          # channels
NW = 64         # windows (8x8 grid of 32x32 patches on 256x256)
T = 1024        # tokens per window (32*32)
NCORES = 8
WPC = NW // NCORES  # windows per core
BF16 = ml_dtypes.bfloat16

_CACHE = {}


def _build_program():
    import concourse.mybir as mybir
    from concourse import bacc
    from concourse.tile import TileContext

    bf16 = mybir.dt.bfloat16
    f32 = mybir.dt.float32

    nc = bacc.Bacc(None, target_bir_lowering=False)
    # c-major [c, i, t] for q_m/k_m; [s, i, k, c] for v (s = token % 128,
    # k = token // 128 within the window)
    qm_d = nc.declare_dram_parameter("qm", [C, WPC, T], bf16, isOutput=False)
    km_d = nc.declare_dram_parameter("km", [C, WPC, T], bf16, isOutput=False)
    v_d = nc.declare_dram_parameter("v", [128, WPC, 8, C], bf16, isOutput=False)
    o_d = nc.declare_dram_parameter("o", [C, WPC, T], bf16, isOutput=True)

    with TileContext(nc) as tc:
        with (
            tc.tile_pool(name="in", bufs=1) as in_pool,
            tc.tile_pool(name="at", bufs=2) as a_pool,
            tc.tile_pool(name="ob", bufs=2) as o_pool,
            tc.tile_pool(name="pa", bufs=2, space="PSUM") as pa_pool,
            tc.tile_pool(name="po", bufs=2, space="PSUM") as po_pool,
        ):
            # persistent SBUF tiles, loaded once (DMA split per window so
            # the pieces spread across queues and window 0 starts early)
            qm_t = in_pool.tile([C, WPC, T], bf16, tag="qm")
            km_t = in_pool.tile([C, WPC, T], bf16, tag="km")
            v_t = in_pool.tile([128, WPC, 8, C], bf16, tag="v")
            o_t = in_pool.tile([C, WPC, T], bf16, tag="o")
            for i in range(WPC):
                nc.sync.dma_start(out=qm_t[:, i], in_=qm_d[:, i])
                nc.sync.dma_start(out=km_t[:, i], in_=km_d[:, i])
                nc.sync.dma_start(out=v_t[:, i], in_=v_d[:, i])

            for i in range(WPC):
                # per s-chunk k: QK matmul -> relu -> AV accumulate into ps_o.
                # relu is split into t-halves running concurrently on the
                # Scalar (h=0) and Vector (h=1) engines so the tensor engine
                # never waits a full [128,1024] relu latency.
                ps_o = po_pool.tile([C, T], f32, tag="pso")
                for k in range(8):
                    ps_a = [
                        pa_pool.tile([128, 512], f32, tag=f"psa{h}", name=f"psa{h}")
                        for h in range(2)
                    ]
                    attn_t = [
                        a_pool.tile([128, 512], bf16, tag=f"attn{h}", name=f"attn{h}")
                        for h in range(2)
                    ]
                    for h in range(2):
                        nc.tensor.matmul(
                            out=ps_a[h],
                            lhsT=km_t[:, i, k * 128:(k + 1) * 128],
                            rhs=qm_t[:, i, h * 512:(h + 1) * 512],
                            start=True,
                            stop=True,
                        )
                    nc.scalar.activation(
                        out=attn_t[0],
                        in_=ps_a[0],
                        func=mybir.ActivationFunctionType.Relu,
                        scale=1.0,
                    )
                    nc.vector.tensor_scalar_max(attn_t[1], ps_a[1], 0.0)
                    for h in range(2):
                        nc.tensor.matmul(
                            out=ps_o[:, h * 512:(h + 1) * 512],
                            lhsT=v_t[:, i, k, :],
                            rhs=attn_t[h],
                            start=(k == 0),
                            stop=(k == 7),
                        )
                if i % 2 == 0:
                    nc.vector.tensor_copy(out=o_t[:, i], in_=ps_o)
                else:
                    nc.scalar.activation(
                        out=o_t[:, i],
                        in_=ps_o,
                        func=mybir.ActivationFunctionType.Copy,
                        scale=1.0,
                    )
                nc.sync.dma_start(out=o_d[:, i], in_=o_t[:, i])

    nc.finalize()
    return nc


def kernel(x, W, bias, _trace=False):
    global LAST_RESULT
    from concourse.bass_utils import run_bass_kernel_spmd

    x = np.asarray(x, dtype=np.float32)
    W = np.asarray(W, dtype=np.float32)
    bias = np.asarray(bias, dtype=np.float32)

    # ---- host prep: windows, qkv, routing, mixing (tiny vs attention) ----
    # xw: [nw, T, c]
    xw = (
        x.reshape(C, 8, 32, 8, 32)
        .transpose(1, 3, 2, 4, 0)
        .reshape(NW, T, C)
    )
    qkv = xw @ W.T + bias  # [nw, T, 3c]
    q, k, v = qkv[..., :C], qkv[..., C:2 * C], qkv[..., 2 * C:]
    q_r = q.mean(axis=1)  # [nw, c]
    k_r = k.mean(axis=1)
    a_r = np.maximum(q_r @ k_r.T, 0.0)  # [nw, nw]
    k_m = np.tensordot(a_r, k, axes=(1, 0))  # [nw, T, c]
    q_m = np.tensordot(a_r, q, axes=(1, 0))

    if "nc" not in _CACHE:
        _CACHE["nc"] = _build_program()
    nc = _CACHE["nc"]

    in_maps = []
    for m in range(NCORES):
        s = slice(m * WPC, (m + 1) * WPC)
        # v: [wpc, T, c] -> [wpc, k, s(128), c] -> [s, wpc, k, c]
        v_s = v[s].reshape(WPC, 8, 128, C).transpose(2, 0, 1, 3)
        in_maps.append({
            "qm": np.ascontiguousarray(q_m[s].transpose(2, 0, 1)).astype(BF16),
            "km": np.ascontiguousarray(k_m[s].transpose(2, 0, 1)).astype(BF16),
            "v": np.ascontiguousarray(v_s).astype(BF16),
        })

    res = run_bass_kernel_spmd(nc, in_maps, list(range(NCORES)), trace=_trace)
    LAST_RESULT = res
    outs = [
        res.results[m]["o"].astype(np.float32).reshape(C, WPC, T)
        for m in range(NCORES)
    ]
    o_cm = np.concatenate(outs, axis=1)  # [c, nw, T]

    # fold back: [c, jh, jw, th, tw] -> [1, c, 256, 256]
    o_img = (
        o_cm.reshape(C, 8, 8, 32, 32)
        .transpose(0, 1, 3, 2, 4)
        .reshape(1, C, 256, 256)
    )
    return o_img.astype(np.float32)


LAST_RESULT = None  # BassKernelResults from the most recent run (for test.py)


# revision 10
# speedup vs baseline: 1.3196x; 1.3196x over previous
"""Window-routed sparse attention on 8 TRN2 NeuronCores.

Sharding: 64 windows x 8 cores = 8 windows/core (embarrassingly parallel).
Host precomputes the tiny routing path (region means, a_r [64,64]) and the
window-mixed q_m/k_m in fp32; each core runs the heavy windowed attention
relu(q_m k_m^T) v for its 8 windows on the Tensor engine in bf16 (f32 PSUM
accumulation). Relu alternates between the Scalar and Vector engines so it
hides under the matmuls; PSUM->SBUF output copies run on GpSimd.
"""

import sys

sys.path.insert(0, "/opt/trn_rl_repo")

import numpy as np
import ml_dtypes

C = 64          # channels
NW = 64         # windows (8x8 grid of 32x32 patches on 256x256)
T = 1024        # tokens per window (32*32)
NCORES = 8
WPC = NW // NCORES  # windows per core
BF16 = ml_dtypes.bfloat16

_CACHE = {}


def _build_program():
    import concourse.mybir as mybir
    from concourse import bacc
    from concourse.tile import TileContext

    bf16 = mybir.dt.bfloat16
    f32 = mybir.dt.float32

    nc = bacc.Bacc(None, target_bir_lowering=False)
    # c-major [c, i, t] for q_m/k_m; [s, i, k, c] for v (s = token % 128,
    # k = token // 128 within the window)
    qm_d = nc.declare_dram_parameter("qm", [C, WPC, T], bf16, isOutput=False)
    km_d = nc.declare_dram_parameter("km", [C, WPC, T], bf16, isOutput=False)
    v_d = nc.declare_dram_parameter("v", [128, WPC, 8, C], bf16, isOutput=False)
    o_d = nc.declare_dram_parameter("o", [C, WPC, T], bf16, isOutput=True)

    with TileContext(nc) as tc:
        with (
            tc.tile_pool(name="in", bufs=1) as in_pool,
            tc.tile_pool(name="at", bufs=3) as a_pool,
            tc.tile_pool(name="ob", bufs=2) as o_pool,
            tc.tile_pool(name="pa", bufs=3, space="PSUM") as pa_pool,
            tc.tile_pool(name="po", bufs=1, space="PSUM") as po_pool,
        ):
            # persistent SBUF tiles, loaded once (DMA split per window so
            # the pieces spread across queues and window 0 starts early)
            qm_t = in_pool.tile([C, WPC, T], bf16, tag="qm")
            km_t = in_pool.tile([C, WPC, T], bf16, tag="km")
            v_t = in_pool.tile([128, WPC, 8, C], bf16, tag="v")
            o_t = in_pool.tile([C, WPC, T], bf16, tag="o")
            for i in range(WPC):
                nc.sync.dma_start(out=qm_t[:, i], in_=qm_d[:, i])
                nc.sync.dma_start(out=km_t[:, i], in_=km_d[:, i])
                nc.sync.dma_start(out=v_t[:, i], in_=v_d[:, i])

            for i in range(WPC):
                # Software-pipelined: QK for s-chunk k+2 issues before AV for
                # chunk k, so the PE never waits on a relu. Relu halves run
                # concurrently on Scalar (t 0:512) and Vector (t 512:1024).
                ps_o = po_pool.tile([C, T], f32, tag="pso")
                attn = {}

                def emit_qk(k):
                    ps_a = pa_pool.tile([128, T], f32, tag="psa", name="psa")
                    at = a_pool.tile([128, T], bf16, tag="attn", name="attn")
                    for h in range(2):
                        nc.tensor.matmul(
                            out=ps_a[:, h * 512:(h + 1) * 512],
                            lhsT=km_t[:, i, k * 128:(k + 1) * 128],
                            rhs=qm_t[:, i, h * 512:(h + 1) * 512],
                            start=True,
                            stop=True,
                        )
                    nc.scalar.activation(
                        out=at[:, 0:512],
                        in_=ps_a[:, 0:512],
                        func=mybir.ActivationFunctionType.Relu,
                        scale=1.0,
                    )
                    nc.vector.tensor_scalar_max(at[:, 512:1024], ps_a[:, 512:1024], 0.0)
                    attn[k] = at

                emit_qk(0)
                emit_qk(1)
                for k in range(8):
                    if k + 2 < 8:
                        emit_qk(k + 2)
                    at = attn.pop(k)
                    for h in range(2):
                        nc.tensor.matmul(
                            out=ps_o[:, h * 512:(h + 1) * 512],
                            lhsT=v_t[:, i, k, :],
                            rhs=at[:, h * 512:(h + 1) * 512],
                            start=(k == 0),
                            stop=(k == 7),
                        )
                if i % 2 == 0:
                    nc.vector.tensor_copy(out=o_t[:, i], in_=ps_o)
                else:
                    nc.scalar.activation(
                        out=o_t[:, i],
                        in_=ps_o,
                        func=mybir.ActivationFunctionType.Copy,
                        scale=1.0,
                    )
                nc.sync.dma_start(out=o_d[:, i], in_=o_t[:, i])

    nc.finalize()
    return nc


def kernel(x, W, bias, _trace=False):
    global LAST_RESULT
    from concourse.bass_utils import run_bass_kernel_spmd

    x = np.asarray(x, dtype=np.float32)
    W = np.asarray(W, dtype=np.float32)
    bias = np.asarray(bias, dtype=np.float32)

    # ---- host prep: windows, qkv, routing, mixing (tiny vs attention) ----
    # xw: [nw, T, c]
    xw = (
        x.reshape(C, 8, 32, 8, 32)
        .transpose(1, 3, 2, 4, 0)
        .reshape(NW, T, C)
    )
    qkv = xw @ W.T + bias  # [nw, T, 3c]
    q, k, v = qkv[..., :C], qkv[..., C:2 * C], qkv[..., 2 * C:]
    q_r = q.mean(axis=1)  # [nw, c]
    k_r = k.mean(axis=1)
    a_r = np.maximum(q_r @ k_r.T, 0.0)  # [nw, nw]
    k_m = np.tensordot(a_r, k, axes=(1, 0))  # [nw, T, c]
    q_m = np.tensordot(a_r, q, axes=(1, 0))

    if "nc" not in _CACHE:
        _CACHE["nc"] = _build_program()
    nc = _CACHE["nc"]

    in_maps = []
    for m in range(NCORES):
        s = slice(m * WPC, (m + 1) * WPC)
        # v: [wpc, T, c] -> [wpc, k, s(128), c] -> [s, wpc, k, c]
        v_s = v[s].reshape(WPC, 8, 128, C).transpose(2, 0, 1, 3)
        in_maps.append({
            "qm": np.ascontiguousarray(q_m[s].transpose(2, 0, 1)).astype(BF16),
            "km": np.ascontiguousarray(k_m[s].transpose(2, 0, 1)).astype(BF16),
            "v": np.ascontiguousarray(v_s).astype(BF16),
        })

    res = run_bass_kernel_spmd(nc, in_maps, list(range(NCORES)), trace=_trace)
    LAST_RESULT = res
    outs = [
        res.results[m]["o"].astype(np.float32).reshape(C, WPC, T)
        for m in range(NCORES)
    ]
    o_cm = np.concatenate(outs, axis=1)  # [c, nw, T]

    # fold back: [c, jh, jw, th, tw] -> [1, c, 256, 256]
    o_img = (
        o_cm.reshape(C, 8, 8, 32, 32)
        .transpose(0, 1, 3, 2, 4)
        .reshape(1, C, 256, 256)
    )
    return o_img.astype(np.float32)


LAST_RESULT = None  # BassKernelResults from the most recent run (for test.py)
